# revision 1
# baseline (speedup 1.0000x reference)
"""GCN layer kernel for 8 trn2 NeuronCores (SPMD, single launch).

Math:  out = D^-1/2 (A+I) D^-1/2 X W^T + b
Key identity: the dense layer commutes with the diagonal scalings:
    out = D^-1/2 (A+I) D^-1/2 (X W^T) + b
so we compute U = X@W^T (tiny) first, then one big matmul A @ (d^-1/2 * U).

Distribution: row-shard A across 8 cores (strip = 1024 rows). Each core:
  phase 1: stream its fp32 strip once from HBM; per 128x128 tile,
           PE-transpose (fp32 transpose mode) -> PSUM -> copy to a
           SBUF-resident bf16 A^T strip (16.8MB, fits in 24MB SBUF);
           simultaneously row-sum the natural chunks on VectorE.
  AllGather (the only collective): 1024 local row sums -> full degree.
  phase 2: d^-1/2 via sqrt+reciprocal+Newton; Y = d^-1/2*U (bf16);
           Z = A^T-tiles^T @ Y accumulated in PSUM over 64 k-tiles;
           out = d^-1/2*(Z + Y_local) + b  (self-loop handled exactly).

A is read from HBM exactly once (33.5MB/core ~ 93us at 360GB/s roofline).
"""

import numpy as np
import ml_dtypes

N = 8192          # nodes
F = 128           # in/out feature dim
NCORES = 8
SR = N // NCORES  # strip rows per core = 1024
P = 128           # partitions / tile edge
IT = SR // P      # 8 row tiles per strip
JT = N // P       # 64 contraction tiles
CH = 2048         # chunk columns for DMA
NCH = N // CH     # 4 chunks per row-tile

_CACHE = {}


def _build_nc():
    import concourse.mybir as mybir
    from concourse import bass
    from concourse.tile import TileContext

    f32 = mybir.dt.float32
    bf16 = mybir.dt.bfloat16
    AF = mybir.ActivationFunctionType

    nc = bass.Bass(num_devices=NCORES)

    A_s = nc.declare_dram_parameter("a_strip", [SR, N], f32, False)
    Xt = nc.declare_dram_parameter("xt_bf", [P, N], bf16, False)       # X^T, bf16
    XtL = nc.declare_dram_parameter("xt_loc", [P, SR], bf16, False)    # local cols of X^T
    Wt = nc.declare_dram_parameter("wt", [P, F], f32, False)           # W^T
    Bb = nc.declare_dram_parameter("b_bc", [P, F], f32, False)         # bias bcast to 128 rows
    Idn = nc.declare_dram_parameter("ident", [P, P], f32, False)
    out = nc.declare_dram_parameter("out", [SR, F], f32, True)

    degL = nc.dram_tensor("deg_local", [IT, P], f32)
    degA = nc.dram_tensor("deg_all", [JT, P], f32, addr_space="Shared")

    with TileContext(nc) as tc:
        with tc.tile_pool(name="const", bufs=1) as constp, \
             tc.tile_pool(name="big", bufs=1) as bigp, \
             tc.tile_pool(name="chunks", bufs=2) as chp, \
             tc.tile_pool(name="small", bufs=1) as smallp, \
             tc.tile_pool(name="outs", bufs=3) as outp, \
             tc.tile_pool(name="trps", bufs=3, space="PSUM") as trps, \
             tc.tile_pool(name="zps", bufs=2, space="PSUM") as zps, \
             tc.tile_pool(name="ups", bufs=2, space="PSUM") as ups:

            # ---- constants / small inputs ----
            ident = constp.tile([P, P], f32)
            nc.sync.dma_start(out=ident[:, :], in_=Idn[:, :])
            wt_sb = constp.tile([P, F], f32)
            nc.sync.dma_start(out=wt_sb[:, :], in_=Wt[:, :])
            bb_sb = constp.tile([P, F], f32)
            nc.sync.dma_start(out=bb_sb[:, :], in_=Bb[:, :])
            wt_bf = constp.tile([P, F], bf16)
            nc.vector.tensor_copy(wt_bf[:, :], wt_sb[:, :])

            xt_sb = bigp.tile([P, N], bf16)
            nc.sync.dma_start(out=xt_sb[:, :], in_=Xt[:, :])
            xtl_sb = constp.tile([P, SR], bf16)
            nc.sync.dma_start(out=xtl_sb[:, :], in_=XtL[:, :])

            # ---- persistent big buffers ----
            At = bigp.tile([P, IT * JT * P], bf16)   # transposed strip, bf16
            Yp = bigp.tile([P, N], bf16)             # U then Y' (scaled), per-jt tiles
            Yloc = bigp.tile([P, SR], f32)           # local U then Y'_local (fp32)
            rsp = smallp.tile([P, IT * NCH], f32)    # row-sum partials
            rs = smallp.tile([P, IT], f32)           # local row sums [p, it]

            # ---- U = X @ W^T  (64 small matmuls; overlaps with A streaming) ----
            for jt in range(JT):
                ups_t = ups.tile([P, F], f32)
                nc.tensor.matmul(
                    ups_t[:, :], xt_sb[:, jt * P:(jt + 1) * P], wt_bf[:, :],
                    start=True, stop=True,
                )
                nc.scalar.copy(Yp[:, jt * F:(jt + 1) * F], ups_t[:, :])
            for it in range(IT):
                ups_t = ups.tile([P, F], f32)
                nc.tensor.matmul(
                    ups_t[:, :], xtl_sb[:, it * P:(it + 1) * P], wt_bf[:, :],
                    start=True, stop=True,
                )
                nc.vector.tensor_copy(Yloc[:, it * F:(it + 1) * F], ups_t[:, :])

            # ---- phase 1: stream A strip; transpose + row-sum ----
            ncopy = 0
            for it in range(IT):
                for g in range(NCH):
                    ch = chp.tile([P, CH], f32)
                    nc.sync.dma_start(
                        out=ch[:, :],
                        in_=A_s[it * P:(it + 1) * P, g * CH:(g + 1) * CH],
                    )
                    nc.vector.tensor_reduce(
                        rsp[:, it * NCH + g:it * NCH + g + 1], ch[:, :],
                        axis=mybir.AxisListType.X, op=mybir.AluOpType.add,
                    )
                    for h in range(4):  # 4 psum banks per chunk, 4 tiles each
                        ps = trps.tile([P, 512], f32)
                        for q in range(4):
                            sub = ch[:, (h * 4 + q) * P:(h * 4 + q + 1) * P]
                            nc.tensor.transpose(
                                ps[:, q * P:(q + 1) * P], sub, ident[:, :],
                            )
                        jt0 = g * 16 + h * 4
                        dest = At[:, (it * JT + jt0) * P:(it * JT + jt0 + 4) * P]
                        # ~5/12 of drains on VectorE, rest on ScalarE
                        if ncopy % 12 < 5:
                            nc.vector.tensor_copy(dest, ps[:, :])
                        else:
                            nc.scalar.copy(dest, ps[:, :])
                        ncopy += 1

            # combine row-sum partials -> rs[:, it]
            for it in range(IT):
                nc.vector.tensor_reduce(
                    rs[:, it:it + 1], rsp[:, it * NCH:(it + 1) * NCH],
                    axis=mybir.AxisListType.X, op=mybir.AluOpType.add,
                )

            # ---- AllGather local row sums ----
            ps8 = zps.tile([IT, P], f32, tag="z")
            nc.tensor.transpose(ps8[:, :], rs[:, :], ident[:, :])
            rsT = smallp.tile([IT, P], f32)
            nc.vector.tensor_copy(rsT[:, :], ps8[:, :])
            nc.sync.dma_start(out=degL[:, :], in_=rsT[:, :])
            nc.gpsimd.collective_compute(
                "AllGather", mybir.AluOpType.bypass,
                replica_groups=[list(range(NCORES))],
                ins=[degL[:, :]], outs=[degA[:, :]],
            )
            deg_sb = smallp.tile([JT, P], f32)
            nc.sync.dma_start(out=deg_sb[:, :], in_=degA[:, :])

            # ---- d^-1/2 (global [64,128] and local [128,8]) ----
            def rsqrt_newton(dst, src, pool, shape):
                # dst = (src+1)^-1/2 with one Newton step to fix sqrt LUT error
                sq = pool.tile(shape, f32)
                nc.scalar.activation(sq, src, AF.Sqrt, bias=1.0)
                r0 = pool.tile(shape, f32)
                nc.vector.reciprocal(r0, sq)
                d1 = pool.tile(shape, f32)
                nc.vector.tensor_scalar_add(d1, src, 1.0)
                t = pool.tile(shape, f32)
                nc.vector.tensor_mul(t, r0, r0)
                nc.vector.tensor_mul(t, t, d1)
                nc.scalar.activation(t, t, AF.Copy, bias=1.5, scale=-0.5)
                nc.vector.tensor_mul(dst, r0, t)

            dinvG = smallp.tile([JT, P], f32)
            rsqrt_newton(dinvG[:, :], deg_sb[:, :], smallp, [JT, P])
            dinvL = smallp.tile([P, IT], f32)
            rsqrt_newton(dinvL[:, :], rs[:, :], smallp, [P, IT])

            # transpose dinvG [64,128] -> dinvT [128,64] via padded PE transpose
            dpad = smallp.tile([P, P], f32)
            nc.vector.memset(dpad[:, :], 0.0)
            nc.vector.tensor_copy(dpad[0:JT, :], dinvG[:, :])
            dps = zps.tile([P, P], f32, tag="z")
            nc.tensor.transpose(dps[:, :], dpad[:, :], ident[:, :])
            dinvT = smallp.tile([P, JT], f32)
            nc.vector.tensor_copy(dinvT[:, :], dps[:, 0:JT])

            # ---- scale: Yp <- dinv * U (bf16, in place); Yloc fp32 ----
            for jt in range(JT):
                nc.vector.tensor_scalar_mul(
                    Yp[:, jt * F:(jt + 1) * F], Yp[:, jt * F:(jt + 1) * F],
                    dinvT[:, jt:jt + 1],
                )
            for it in range(IT):
                nc.vector.tensor_scalar_mul(
                    Yloc[:, it * F:(it + 1) * F], Yloc[:, it * F:(it + 1) * F],
                    dinvL[:, it:it + 1],
                )

            # ---- phase 2: Z = A_strip @ Y ; epilogue ----
            for it in range(IT):
                zp = zps.tile([P, F], f32, tag="z")
                for jt in range(JT):
                    nc.tensor.matmul(
                        zp[:, :],
                        At[:, (it * JT + jt) * P:(it * JT + jt + 1) * P],
                        Yp[:, jt * F:(jt + 1) * F],
                        start=(jt == 0), stop=(jt == JT - 1),
                    )
                t1 = outp.tile([P, F], f32)
                nc.vector.tensor_add(t1[:, :], zp[:, :], Yloc[:, it * F:(it + 1) * F])
                nc.vector.tensor_scalar_mul(t1[:, :], t1[:, :], dinvL[:, it:it + 1])
                nc.vector.tensor_add(t1[:, :], t1[:, :], bb_sb[:, :])
                nc.sync.dma_start(out=out[it * P:(it + 1) * P, :], in_=t1[:, :])

    return nc


_NO_SPLIT_TYPES = ("InstEventSemaphore", "InstSemaphore", "InstTrigger")


def _split_drain_waits(nc, max_waits=1):
    """This walrus build only encodes one sem-wait per instruction; hoist
    extras onto preceding same-engine NOPs (monotonic sems => equivalent)."""
    import concourse.mybir as mybir
    for fn in nc.m.functions:
        for blk in fn.blocks:
            newlist = []
            for ins in blk.instructions:
                si = getattr(ins, "sync_info", None)
                tname = type(ins).__name__
                if si is not None and si.on_wait and len(si.on_wait) > max_waits \
                        and not any(tname.startswith(t) for t in _NO_SPLIT_TYPES):
                    waits = list(si.on_wait)
                    for j, w in enumerate(waits[max_waits:]):
                        newlist.append(mybir.InstNoOp(
                            name=f"{ins.name}-w{j}", engine=ins.engine,
                            ins=[], outs=[],
                            sync_info=mybir.SyncInfo(on_wait=[w], on_update=[]),
                        ))
                    si.on_wait = waits[:max_waits]
                newlist.append(ins)
            blk.instructions[:] = newlist


def _get_nc():
    if "nc" not in _CACHE:
        nc = _build_nc()
        _split_drain_waits(nc)
        _CACHE["nc"] = nc
    return _CACHE["nc"]


def _make_in_maps(X, A, W, b):
    bf16 = ml_dtypes.bfloat16
    X = np.ascontiguousarray(np.asarray(X, dtype=np.float32))
    A = np.ascontiguousarray(np.asarray(A, dtype=np.float32))
    W = np.ascontiguousarray(np.asarray(W, dtype=np.float32))
    b = np.ascontiguousarray(np.asarray(b, dtype=np.float32))
    Xt_bf = np.ascontiguousarray(X.T).astype(bf16)
    Wt = np.ascontiguousarray(W.T)
    Bb = np.ascontiguousarray(np.tile(b[None, :], (P, 1)))
    Idn = np.eye(P, dtype=np.float32)
    in_maps = []
    for c in range(NCORES):
        in_maps.append({
            "a_strip": np.ascontiguousarray(A[c * SR:(c + 1) * SR, :]),
            "xt_bf": Xt_bf,
            "xt_loc": np.ascontiguousarray(Xt_bf[:, c * SR:(c + 1) * SR]),
            "wt": Wt,
            "b_bc": Bb,
            "ident": Idn,
        })
    return in_maps


def _install_ntff_hook():
    """This image's antenv lacks axon_hooks; synthesize it so trace=True
    can reach the terminal's NTFF capture via the libaxon ctypes hook."""
    import sys
    import types
    if "antenv.axon_hooks" in sys.modules:
        return
    try:
        from trn_agent_boot.trn_boot import _ntff_profile_via_ctypes
        hook = _ntff_profile_via_ctypes("/opt/axon/libaxon_pjrt.so")
    except Exception:
        hook = None
    mod = types.ModuleType("antenv.axon_hooks")
    mod._hook = hook
    mod.get_axon_ntff_profile_hook = lambda: mod._hook
    def _set(h):
        mod._hook = h
    mod.set_axon_ntff_profile_hook = _set
    sys.modules["antenv.axon_hooks"] = mod
    import antenv
    antenv.axon_hooks = mod
    # the artifact upload needs a bucket this sandbox doesn't have
    import concourse.bass_utils as bu
    bu.upload_artifacts = lambda tmpdir: f"local:{tmpdir}"


def run(X, A, W, b, trace=False, **trace_kwargs):
    """Run on hardware; returns (output, BassKernelResults)."""
    from concourse.bass_utils import run_bass_kernel_spmd
    if trace:
        _install_ntff_hook()
    nc = _get_nc()
    in_maps = _make_in_maps(X, A, W, b)
    res = run_bass_kernel_spmd(nc, in_maps, list(range(NCORES)),
                               trace=trace, **trace_kwargs)
    outs = [np.asarray(res.results[c]["out"], dtype=np.float32)
            for c in range(NCORES)]
    return np.concatenate(outs, axis=0), res


def kernel(X, A, W, b):
    out, _ = run(X, A, W, b, trace=False)
    return out



# revision 6
# speedup vs baseline: 1.7909x; 1.7909x over previous
"""GCN layer kernel for 8 trn2 NeuronCores (SPMD, single launch).

Math:  out = D^-1/2 (A+I) D^-1/2 X W^T + b
     = S A S U + S^2 U + b,   S = diag(s), s = (rowsum(A)+1)^-1/2, U = X W^T

Distribution: row-shard A across 8 cores (strip = 1024 rows = local i's).
Host prep is layout/dtype only (as the baseline already did for X): each
core receives its strip of A^T as a bf16 "SBUF image"
  at_img[p, w*JT*IW + jt*IW + i] = A[c*1024 + w*IW + i, jt*128 + p]
(WV waves over the local i range). No fp32 on-chip transposes; A is
streamed from HBM exactly once as bf16 (16.8MB/core).

Per core:
  stream at_img -> SBUF (NDMA DMAs). While streaming:
    U = X@W^T (64 small matmuls from X^T bf16, drained to SBUF bf16)
    degree: per jt-pair VectorE pair-add halves the data, then a
    ones^T-stationary matmul partition-reduce accumulates deg[1, IW]
    per wave on TensorE.
  per wave w: local s = rsqrt(deg+1); M = ones x s outer (bcast);
    E = UlocT*M^2 + b; AllGather wave-w degrees (the only collectives).
    Wave 0's AllGather overlaps wave 1's streaming; Z matmuls for
    wave-0-unlocked j-tiles overlap wave 1's AllGather.
  Z^T[f,i] = sum_j U[j,f]*s_j*A[i_loc,j]: per (jt, i-region) one matmul
    with 512-wide moving operand, accumulating WV [128,512] PSUM chains.
  outT = Z^T*M + E; output returned transposed; host transposes at gather.
"""

import numpy as np
import ml_dtypes

N = 8192          # nodes
F = 128           # in/out feature dim
NCORES = 8
SR = N // NCORES  # strip rows per core = 1024 (local i's)
P = 128           # partitions / tile edge
JT = N // P       # 64 j tiles (contraction)
WV = 2            # degree/collective waves over the local i range
IW = SR // WV     # i columns per wave = 512
QW = IW // P      # j-tiles unlocked per wave per rank chunk = 4
GR = NCORES * QW  # gathered-degree rows per wave = 32
NDMA = 16         # stream DMAs (1MB each)

_CACHE = {}


def _build_nc():
    import concourse.mybir as mybir
    from concourse import bass
    from concourse.tile import TileContext

    f32 = mybir.dt.float32
    bf16 = mybir.dt.bfloat16
    AF = mybir.ActivationFunctionType

    nc = bass.Bass(num_devices=NCORES)

    At_d = nc.declare_dram_parameter("at_img", [P, JT * SR], bf16, False)
    Xt = nc.declare_dram_parameter("xt_bf", [P, N], bf16, False)      # X^T bf16
    XtL = nc.declare_dram_parameter("xt_loc", [P, SR], bf16, False)   # local cols
    Wt = nc.declare_dram_parameter("wt", [P, F], f32, False)          # W^T
    Bp = nc.declare_dram_parameter("b_pc", [P, 1], f32, False)        # bias col
    Idn = nc.declare_dram_parameter("ident", [P, P], f32, False)
    outT = nc.declare_dram_parameter("outT", [P, SR], f32, True)      # out^T

    degL = [nc.dram_tensor(f"deg_local{w}", [1, IW], f32) for w in range(WV)]
    degA = [nc.dram_tensor(f"deg_all{w}", [GR, P], f32,
                           addr_space="Shared") for w in range(WV)]

    with TileContext(nc) as tc:
        with tc.tile_pool(name="const", bufs=1) as constp, \
             tc.tile_pool(name="big", bufs=1) as bigp, \
             tc.tile_pool(name="small", bufs=1) as smallp, \
             tc.tile_pool(name="vps", bufs=2) as vpsp, \
             tc.tile_pool(name="ups", bufs=2, space="PSUM") as ups, \
             tc.tile_pool(name="degps", bufs=2, space="PSUM") as degps, \
             tc.tile_pool(name="zps", bufs=2, space="PSUM") as zps:

            # ---- constants / small inputs ----
            ident = constp.tile([P, P], f32)
            nc.sync.dma_start(out=ident[:, :], in_=Idn[:, :])
            wt_sb = constp.tile([P, F], f32)
            nc.sync.dma_start(out=wt_sb[:, :], in_=Wt[:, :])
            bp_sb = constp.tile([P, 1], f32)
            nc.sync.dma_start(out=bp_sb[:, :], in_=Bp[:, :])
            wt_bf = constp.tile([P, F], bf16)
            nc.vector.tensor_copy(wt_bf[:, :], wt_sb[:, :])
            ones_c = constp.tile([P, 1], bf16)
            nc.vector.memset(ones_c[:, :], 1.0)
            ones_r = constp.tile([1, P], f32)
            nc.vector.memset(ones_r[:, :], 1.0)

            xt_sb = bigp.tile([P, N], bf16)
            nc.sync.dma_start(out=xt_sb[:, :], in_=Xt[:, :])
            xtl_sb = constp.tile([P, SR], bf16)
            nc.sync.dma_start(out=xtl_sb[:, :], in_=XtL[:, :])

            # ---- persistent big buffers ----
            At = bigp.tile([P, JT * SR], bf16)   # A^T strip image
            Up = bigp.tile([P, N], bf16)         # U = X@W^T tiles [j, f]
            UlocT = bigp.tile([P, SR], f32)      # U^T local [f, i] -> E
            M = bigp.tile([P, SR], f32)          # s_i bcast over f
            M2 = bigp.tile([P, SR], f32)
            outT_sb = bigp.tile([P, SR], f32)
            sL = smallp.tile([1, SR], f32)       # local s
            degL_sb = smallp.tile([1, SR], f32)

            # ---- stream A^T image ----
            CDMA = JT * SR // NDMA
            for g in range(NDMA):
                nc.sync.dma_start(
                    out=At[:, g * CDMA:(g + 1) * CDMA],
                    in_=At_d[:, g * CDMA:(g + 1) * CDMA],
                )

            # ---- U = X @ W^T (overlaps stream; TensorE + Scalar drains) ----
            for jt in range(JT):
                up_t = ups.tile([P, F], f32, tag="u")
                nc.tensor.matmul(
                    up_t[:, :], xt_sb[:, jt * P:(jt + 1) * P], wt_bf[:, :],
                    start=True, stop=True,
                )
                nc.scalar.copy(Up[:, jt * F:(jt + 1) * F], up_t[:, :])

            # ---- U^T local [f, i] (wt stationary, xt_loc moving) ----
            for h in range(SR // 512):
                ul_t = ups.tile([P, 512], f32, tag="u")
                nc.tensor.matmul(
                    ul_t[:, :], wt_bf[:, :], xtl_sb[:, h * 512:(h + 1) * 512],
                    start=True, stop=True,
                )
                nc.scalar.copy(UlocT[:, h * 512:(h + 1) * 512], ul_t[:, :])

            def rsqrt_newton(dst, src, shape):
                # dst = (src+1)^-1/2 with one Newton step to fix sqrt LUT err
                sq = vpsp.tile(shape, f32, tag="nt0", bufs=1)
                nc.scalar.activation(sq, src, AF.Sqrt, bias=1.0)
                r0 = vpsp.tile(shape, f32, tag="nt1", bufs=1)
                nc.vector.reciprocal(r0, sq)
                d1 = vpsp.tile(shape, f32, tag="nt2", bufs=1)
                nc.vector.tensor_scalar_add(d1, src, 1.0)
                t = vpsp.tile(shape, f32, tag="nt3", bufs=1)
                nc.vector.tensor_mul(t, r0, r0)
                nc.vector.tensor_mul(t, t, d1)
                nc.scalar.activation(t, t, AF.Copy, bias=1.5, scale=-0.5)
                nc.vector.tensor_mul(dst, r0, t)

            # ---- per wave: degree reduce, local s/M/E, AllGather ----
            for w in range(WV):
                dg = degps.tile([1, IW], f32, tag="dg")
                for k in range(JT // 2):
                    jt0, jt1 = 2 * k, 2 * k + 1
                    vp = vpsp.tile([P, IW], bf16, tag="vp")
                    nc.vector.tensor_add(
                        vp[:, :],
                        At[:, (w * JT + jt0) * IW:(w * JT + jt0 + 1) * IW],
                        At[:, (w * JT + jt1) * IW:(w * JT + jt1 + 1) * IW],
                    )
                    nc.tensor.matmul(
                        dg[:, :], ones_c[:, :], vp[:, :],
                        start=(k == 0), stop=(k == JT // 2 - 1),
                    )
                # drain on Scalar (Vector is busy with next wave's pair-adds)
                nc.scalar.copy(degL_sb[0:1, w * IW:(w + 1) * IW], dg[:, :])
                nc.sync.dma_start(out=degL[w][:, :],
                                  in_=degL_sb[0:1, w * IW:(w + 1) * IW])
                # local rsqrt + M outer-product + E (pre-collective work)
                rsqrt_newton(sL[0:1, w * IW:(w + 1) * IW],
                             degL_sb[0:1, w * IW:(w + 1) * IW], [1, IW])
                mp = ups.tile([P, IW], f32, tag="u")
                nc.tensor.matmul(
                    mp[:, :], ones_r[:, :], sL[0:1, w * IW:(w + 1) * IW],
                    start=True, stop=True,
                )
                nc.vector.tensor_copy(M[:, w * IW:(w + 1) * IW], mp[:, :])
                nc.vector.tensor_mul(M2[:, w * IW:(w + 1) * IW],
                                     M[:, w * IW:(w + 1) * IW],
                                     M[:, w * IW:(w + 1) * IW])
                # E = UlocT*M2 + b  (in place on UlocT)
                nc.vector.tensor_mul(UlocT[:, w * IW:(w + 1) * IW],
                                     UlocT[:, w * IW:(w + 1) * IW],
                                     M2[:, w * IW:(w + 1) * IW])
                nc.vector.tensor_scalar_add(UlocT[:, w * IW:(w + 1) * IW],
                                            UlocT[:, w * IW:(w + 1) * IW],
                                            bp_sb[:, 0:1])
                # the only collective
                nc.gpsimd.collective_compute(
                    "AllGather", mybir.AluOpType.bypass,
                    replica_groups=[list(range(NCORES))],
                    ins=[degL[w][:, :]], outs=[degA[w][:, :]],
                )

            # ---- per wave (post-collective): global s, scale Y, Z chains ----
            zp = [zps.tile([P, IW], f32, tag="z", name=f"zp{h}")
                  for h in range(WV)]
            for w in range(WV):
                gdeg = smallp.tile([GR, P], f32, name=f"gdeg{w}")
                nc.sync.dma_start(out=gdeg[:, :], in_=degA[w][:, :])
                sG = smallp.tile([GR, P], f32, name=f"sG{w}")
                rsqrt_newton(sG[:, :], gdeg[:, :], [GR, P])
                dps = zps.tile([P, GR], f32, tag="tp")
                nc.tensor.transpose(dps[:, :], sG[:, :], ident[0:GR, 0:GR])
                dT = smallp.tile([P, GR], f32, name=f"dinvT{w}")
                nc.vector.tensor_copy(dT[:, :], dps[:, :])

                for c in range(NCORES):
                    for q in range(QW):
                        jt = c * (JT // NCORES) + w * QW + q
                        k = c * QW + q
                        nc.vector.tensor_scalar_mul(
                            Up[:, jt * F:(jt + 1) * F],
                            Up[:, jt * F:(jt + 1) * F],
                            dT[:, k:k + 1],
                        )
                        first = (w == 0 and c == 0 and q == 0)
                        last = (w == WV - 1 and c == NCORES - 1
                                and q == QW - 1)
                        for h in range(WV):
                            nc.tensor.matmul(
                                zp[h][:, :],
                                Up[:, jt * F:(jt + 1) * F],
                                At[:, (h * JT + jt) * IW:(h * JT + jt + 1) * IW],
                                start=first, stop=last,
                            )

            # ---- epilogue: outT = Z^T*M + E ; DMA out ----
            for h in range(WV):
                nc.vector.tensor_mul(outT_sb[:, h * IW:(h + 1) * IW],
                                     zp[h][:, :], M[:, h * IW:(h + 1) * IW])
                nc.vector.tensor_add(outT_sb[:, h * IW:(h + 1) * IW],
                                     outT_sb[:, h * IW:(h + 1) * IW],
                                     UlocT[:, h * IW:(h + 1) * IW])
                nc.sync.dma_start(out=outT[:, h * IW:(h + 1) * IW],
                                  in_=outT_sb[:, h * IW:(h + 1) * IW])

    return nc


_NO_SPLIT_TYPES = ("InstEventSemaphore", "InstSemaphore", "InstTrigger")


def _split_drain_waits(nc, max_waits=1):
    """This walrus build only encodes one sem-wait per instruction; hoist
    extras onto preceding same-engine NOPs (monotonic sems => equivalent)."""
    import concourse.mybir as mybir
    for fn in nc.m.functions:
        for blk in fn.blocks:
            newlist = []
            for ins in blk.instructions:
                si = getattr(ins, "sync_info", None)
                tname = type(ins).__name__
                if si is not None and si.on_wait and len(si.on_wait) > max_waits \
                        and not any(tname.startswith(t) for t in _NO_SPLIT_TYPES):
                    waits = list(si.on_wait)
                    for j, w in enumerate(waits[max_waits:]):
                        newlist.append(mybir.InstNoOp(
                            name=f"{ins.name}-w{j}", engine=ins.engine,
                            ins=[], outs=[],
                            sync_info=mybir.SyncInfo(on_wait=[w], on_update=[]),
                        ))
                    si.on_wait = waits[:max_waits]
                newlist.append(ins)
            blk.instructions[:] = newlist


def _get_nc():
    if "nc" not in _CACHE:
        nc = _build_nc()
        _split_drain_waits(nc)
        _CACHE["nc"] = nc
    return _CACHE["nc"]


def _make_in_maps(X, A, W, b):
    bf16 = ml_dtypes.bfloat16
    X = np.ascontiguousarray(np.asarray(X, dtype=np.float32))
    A = np.asarray(A, dtype=np.float32)
    Wm = np.ascontiguousarray(np.asarray(W, dtype=np.float32))
    b = np.ascontiguousarray(np.asarray(b, dtype=np.float32))
    Xt_bf = np.ascontiguousarray(X.T).astype(bf16)
    Wt = np.ascontiguousarray(Wm.T)
    Bp = np.ascontiguousarray(b[:, None])
    Idn = np.eye(P, dtype=np.float32)
    A_bf = A.astype(bf16)
    in_maps = []
    for c in range(NCORES):
        strip = A_bf[c * SR:(c + 1) * SR, :]
        img = np.ascontiguousarray(
            strip.reshape(WV, IW, JT, P).transpose(3, 0, 2, 1)
        ).reshape(P, JT * SR)
        in_maps.append({
            "at_img": img,
            "xt_bf": Xt_bf,
            "xt_loc": np.ascontiguousarray(Xt_bf[:, c * SR:(c + 1) * SR]),
            "wt": Wt,
            "b_pc": Bp,
            "ident": Idn,
        })
    return in_maps


def _install_ntff_hook():
    """This image's antenv lacks axon_hooks; synthesize it so trace=True
    can reach the terminal's NTFF capture via the libaxon ctypes hook."""
    import sys
    import types
    if "antenv.axon_hooks" in sys.modules:
        return
    try:
        from trn_agent_boot.trn_boot import _ntff_profile_via_ctypes
        hook = _ntff_profile_via_ctypes("/opt/axon/libaxon_pjrt.so")
    except Exception:
        hook = None
    mod = types.ModuleType("antenv.axon_hooks")
    mod._hook = hook
    mod.get_axon_ntff_profile_hook = lambda: mod._hook
    def _set(h):
        mod._hook = h
    mod.set_axon_ntff_profile_hook = _set
    sys.modules["antenv.axon_hooks"] = mod
    import antenv
    antenv.axon_hooks = mod
    # the artifact upload needs a bucket this sandbox doesn't have
    import concourse.bass_utils as bu
    bu.upload_artifacts = lambda tmpdir: f"local:{tmpdir}"


def _gather_out(results):
    out = np.empty((N, F), dtype=np.float32)
    for c in range(NCORES):
        out[c * SR:(c + 1) * SR, :] = \
            np.asarray(results[c], dtype=np.float32).T
    return out


def run(X, A, W, b, trace=False, **trace_kwargs):
    """Run on hardware; returns (output, BassKernelResults)."""
    from concourse.bass_utils import run_bass_kernel_spmd
    if trace:
        _install_ntff_hook()
    nc = _get_nc()
    in_maps = _make_in_maps(X, A, W, b)
    res = run_bass_kernel_spmd(nc, in_maps, list(range(NCORES)),
                               trace=trace, **trace_kwargs)
    out = _gather_out([res.results[c]["outT"] for c in range(NCORES)])
    return out, res


def kernel(X, A, W, b):
    out, _ = run(X, A, W, b, trace=False)
    return out
